# revision 1
# baseline (speedup 1.0000x reference)
"""Depthwise causal conv1d (K=4, dilation=1) on 8 TRN2 NeuronCores.

Reference: x [B=8, T=4096, C=1024] f32, W [4, 1, 1024] f32 (WIO layout),
y[b, t, c] = sum_k W[k, 0, c] * x[b, t - 3 + k, c]  (zero left-pad).

Sharding: pure batch data-parallel — core i computes batch i. On the host we
pre-transpose each batch slice to [C, T] (contiguous) so that on-chip the
channel dim sits on SBUF partitions (the per-channel weight becomes a
per-partition scalar operand) and the causal time shifts become free-dim
offsets. The device writes y in [C, T] layout; the host transposes back.

Per-core compute, per channel-group g (8 groups of 128 channels) and time
tile: load x tile [128, TT+3] (3-col halo, zero for t<0), then
  ScalarE: y = x[:, 3:] * W3            (activation Copy with per-partition scale)
  VectorE: y = (x[:, 2:] * W2) + y      (scalar_tensor_tensor) x3 taps
and DMA y back out.
"""

import numpy as np

B, T, C = 8, 4096, 1024
KTAPS = 4
HALO = KTAPS - 1
CG = 128  # channels per partition-group
N_CORES = 8

# module-level stash so test.py can read profiling info
last_results = None


def _build_program(c=C, t=T, tt=2048, xbufs=4, ybufs=4):
    import concourse.bass as bass  # noqa: F401
    import concourse.tile as tile
    from concourse import bacc, mybir

    nc = bacc.Bacc(
        "TRN2",
        target_bir_lowering=False,
        debug=False,
        enable_asserts=False,
        num_devices=N_CORES,
    )
    n_groups = c // CG
    n_tt = t // tt
    x_ap = nc.dram_tensor("x_t", [c, t], mybir.dt.float32, kind="ExternalInput").ap()
    w_ap = nc.dram_tensor(
        "w_t", [CG, n_groups * KTAPS], mybir.dt.float32, kind="ExternalInput"
    ).ap()
    out_ap = nc.dram_tensor("out", [c, t], mybir.dt.float32, kind="ExternalOutput").ap()

    mult = mybir.AluOpType.mult
    add = mybir.AluOpType.add

    with tile.TileContext(nc) as tc:
        with (
            tc.tile_pool(name="wpool", bufs=1) as wpool,
            tc.tile_pool(name="xpool", bufs=xbufs) as xpool,
            tc.tile_pool(name="ypool", bufs=ybufs) as ypool,
        ):
            wt = wpool.tile([CG, n_groups * KTAPS], mybir.dt.float32)
            nc.sync.dma_start(wt[:], w_ap[:])
            for g in range(n_groups):
                r0, r1 = g * CG, (g + 1) * CG
                for j in range(n_tt):
                    t0 = j * tt
                    xt = xpool.tile([CG, tt + HALO], mybir.dt.float32)
                    if j == 0:
                        nc.vector.memset(xt[:, 0:HALO], 0.0)
                        nc.sync.dma_start(xt[:, HALO : HALO + tt], x_ap[r0:r1, 0:tt])
                    else:
                        nc.sync.dma_start(xt[:], x_ap[r0:r1, t0 - HALO : t0 + tt])
                    yt = ypool.tile([CG, tt], mybir.dt.float32)
                    wcol = g * KTAPS
                    # seed with the last tap on ScalarE (keeps VectorE at 3 ops)
                    nc.scalar.mul(
                        yt[:], xt[:, HALO : HALO + tt], wt[:, wcol + 3 : wcol + 4]
                    )
                    for k in (2, 1, 0):
                        nc.vector.scalar_tensor_tensor(
                            yt[:],
                            xt[:, k : k + tt],
                            wt[:, wcol + k : wcol + k + 1],
                            yt[:],
                            op0=mult,
                            op1=add,
                        )
                    nc.sync.dma_start(out_ap[r0:r1, t0 : t0 + tt], yt[:])
    nc.compile()
    return nc


def _prep_weights(W: np.ndarray, c=C) -> np.ndarray:
    # wt[p, g*KTAPS + k] = W[k, 0, g*CG + p]
    n_groups = c // CG
    wk = W.reshape(KTAPS, n_groups, CG)  # [k, g, p]
    return np.ascontiguousarray(wk.transpose(2, 1, 0).reshape(CG, n_groups * KTAPS))


def kernel(x: np.ndarray, W: np.ndarray) -> np.ndarray:
    global last_results
    from concourse.bass_utils import run_bass_kernel_spmd

    x = np.asarray(x, dtype=np.float32)
    W = np.asarray(W, dtype=np.float32)
    assert x.shape == (B, T, C) and W.shape == (KTAPS, 1, C)

    nc = _build_program()
    wt = _prep_weights(W)
    in_maps = [
        {
            "x_t": np.ascontiguousarray(x[i].T),  # [C, T]
            "w_t": wt,
        }
        for i in range(N_CORES)
    ]
    import os

    trace = bool(os.environ.get("BASS_TRACE"))
    res = run_bass_kernel_spmd(
        nc, in_maps, core_ids=list(range(N_CORES)), trace=trace
    )
    last_results = res
    y = np.stack([np.asarray(res.results[i]["out"]).T for i in range(N_CORES)])
    return np.ascontiguousarray(y.astype(np.float32))


# revision 2
# speedup vs baseline: 1.0600x; 1.0600x over previous
"""Depthwise causal conv1d (K=4, dilation=1) on 8 TRN2 NeuronCores.

Reference: x [B=8, T=4096, C=1024] f32, W [4, 1, 1024] f32 (WIO layout),
y[b, t, c] = sum_k W[k, 0, c] * x[b, t - 3 + k, c]  (zero left-pad).

Sharding: pure batch data-parallel — core i computes batch i. On the host we
pre-transpose each batch slice to [C, T] (contiguous) so that on-chip the
channel dim sits on SBUF partitions (the per-channel weight becomes a
per-partition scalar operand) and the causal time shifts become free-dim
offsets. The device writes y in [C, T] layout; the host transposes back.

Per-core compute, per channel-group g (8 groups of 128 channels) and time
tile: load x tile [128, TT+3] (3-col halo, zero for t<0). Work is split
between two fp32-exact paths to balance engines (VectorE alone is the
bottleneck at ~122us vs the ~94us HBM roofline):
 - DVE path: ScalarE seeds y = x3*W3 (per-partition scale), then 3x
   VectorE scalar_tensor_tensor accumulates the other taps.
 - PE path: per 512-col chunk, 4 accumulating matmuls with diagonal
   [128x128] weight matrices (host-built) shift+scale+sum all taps into
   PSUM; ScalarE evicts PSUM -> SBUF.
"""

import numpy as np

B, T, C = 8, 4096, 1024
KTAPS = 4
HALO = KTAPS - 1
CG = 128  # channels per partition-group
N_CORES = 8
MM_N = 512  # fp32 moving-operand max free dim / one PSUM bank

# module-level stash so test.py can read profiling info
last_results = None


def _build_program(c=C, t=T, tt=2048, xbufs=5, ybufs=5, psbufs=8, pe_mod=(3, 1)):
    import concourse.bass as bass  # noqa: F401
    import concourse.tile as tile
    from concourse import bacc, mybir

    nc = bacc.Bacc(
        "TRN2",
        target_bir_lowering=False,
        debug=False,
        enable_asserts=False,
        num_devices=N_CORES,
    )
    n_groups = c // CG
    n_tt = t // tt
    f32 = mybir.dt.float32
    x_ap = nc.dram_tensor("x_t", [c, t], f32, kind="ExternalInput").ap()
    w_ap = nc.dram_tensor("w_t", [CG, n_groups * KTAPS], f32, kind="ExternalInput").ap()
    wd_ap = nc.dram_tensor(
        "w_diag", [CG, n_groups * KTAPS * CG], f32, kind="ExternalInput"
    ).ap()
    out_ap = nc.dram_tensor("out", [c, t], f32, kind="ExternalOutput").ap()

    mult = mybir.AluOpType.mult
    add = mybir.AluOpType.add

    with tile.TileContext(nc) as tc:
        with (
            tc.tile_pool(name="wpool", bufs=1) as wpool,
            tc.tile_pool(name="xpool", bufs=xbufs) as xpool,
            tc.tile_pool(name="ypool", bufs=ybufs) as ypool,
            tc.tile_pool(name="pspool", bufs=psbufs, space="PSUM") as pspool,
        ):
            wt = wpool.tile([CG, n_groups * KTAPS], f32)
            nc.sync.dma_start(wt[:], w_ap[:])
            wd = wpool.tile([CG, n_groups * KTAPS * CG], f32)
            nc.sync.dma_start(wd[:], wd_ap[:])

            for g in range(n_groups):
                r0, r1 = g * CG, (g + 1) * CG
                for j in range(n_tt):
                    idx = g * n_tt + j
                    t0 = j * tt
                    xt = xpool.tile([CG, tt + HALO], f32)
                    if j == 0:
                        nc.vector.memset(xt[:, 0:HALO], 0.0)
                        nc.sync.dma_start(xt[:, HALO : HALO + tt], x_ap[r0:r1, 0:tt])
                    else:
                        nc.sync.dma_start(xt[:], x_ap[r0:r1, t0 - HALO : t0 + tt])
                    yt = ypool.tile([CG, tt], f32)
                    on_pe = pe_mod is not None and idx % pe_mod[0] == pe_mod[1]
                    if on_pe:
                        for c0 in range(0, tt, MM_N):
                            ps = pspool.tile([CG, MM_N], f32)
                            for ki, k in enumerate((3, 2, 1, 0)):
                                dcol = (g * KTAPS + k) * CG
                                nc.tensor.matmul(
                                    ps[:],
                                    wd[:, dcol : dcol + CG],
                                    xt[:, c0 + k : c0 + k + MM_N],
                                    start=(ki == 0),
                                    stop=(ki == KTAPS - 1),
                                )
                            nc.scalar.copy(yt[:, c0 : c0 + MM_N], ps[:])
                    else:
                        wcol = g * KTAPS
                        # seed with the last tap on ScalarE (keeps VectorE at 3 ops)
                        nc.scalar.mul(
                            yt[:], xt[:, HALO : HALO + tt], wt[:, wcol + 3 : wcol + 4]
                        )
                        for k in (2, 1, 0):
                            nc.vector.scalar_tensor_tensor(
                                yt[:],
                                xt[:, k : k + tt],
                                wt[:, wcol + k : wcol + k + 1],
                                yt[:],
                                op0=mult,
                                op1=add,
                            )
                    nc.sync.dma_start(out_ap[r0:r1, t0 : t0 + tt], yt[:])
    nc.compile()
    return nc


def _prep_weights(W: np.ndarray, c=C) -> np.ndarray:
    # wt[p, g*KTAPS + k] = W[k, 0, g*CG + p]
    n_groups = c // CG
    wk = W.reshape(KTAPS, n_groups, CG)  # [k, g, p]
    return np.ascontiguousarray(wk.transpose(2, 1, 0).reshape(CG, n_groups * KTAPS))


def _prep_diag_weights(W: np.ndarray, c=C) -> np.ndarray:
    # wd[i, (g*KTAPS + k)*CG + j] = W[k, 0, g*CG + i] if i == j else 0
    n_groups = c // CG
    wk = W.reshape(KTAPS, n_groups, CG)  # [k, g, i]
    wd = np.zeros((CG, n_groups * KTAPS * CG), dtype=np.float32)
    eye = np.eye(CG, dtype=np.float32)
    for g in range(n_groups):
        for k in range(KTAPS):
            blk = (g * KTAPS + k) * CG
            wd[:, blk : blk + CG] = eye * wk[k, g][:, None]
    return wd


def kernel(x: np.ndarray, W: np.ndarray) -> np.ndarray:
    global last_results
    from concourse.bass_utils import run_bass_kernel_spmd

    x = np.asarray(x, dtype=np.float32)
    W = np.asarray(W, dtype=np.float32)
    assert x.shape == (B, T, C) and W.shape == (KTAPS, 1, C)

    nc = _build_program()
    wt = _prep_weights(W)
    wd = _prep_diag_weights(W)
    in_maps = [
        {
            "x_t": np.ascontiguousarray(x[i].T),  # [C, T]
            "w_t": wt,
            "w_diag": wd,
        }
        for i in range(N_CORES)
    ]
    import os

    trace = bool(os.environ.get("BASS_TRACE"))
    res = run_bass_kernel_spmd(
        nc, in_maps, core_ids=list(range(N_CORES)), trace=trace
    )
    last_results = res
    y = np.stack([np.asarray(res.results[i]["out"]).T for i in range(N_CORES)])
    return np.ascontiguousarray(y.astype(np.float32))
